# revision 7
# baseline (speedup 1.0000x reference)
"""Trainium2 Bass kernel v2 for DifferentiableHardKMeans (vq_codebook).

Math (per pixel p, cluster k):
    t[k,p]  = c2[k] - 2*c[k]@x[p]          (PE matmul, fp32r or fp32)
    u[p,k]  = t[p,k] + x2[p]
    s[p,k]  = g[p,k] - sqrt(u[p,k])
    idx[p]  = argmax_k s[p,k]
Output one-hot is expanded host-side from idx (exact 0.0/1.0 values,
matching the reference's y_hard - sg(y_soft) + y_soft identity).

Layout: per core (batch b -> core b), pixels processed in blocks of 512
(4 tiles of 128). Pixel i = 512*blk + 128*il + r lives on partition r,
free slot il. g is host-shuffled so each partition reads 1KB contiguous;
x is read with a strided AP (1KB runs). idx accumulates in SBUF and is
written once at the end as [128, nblk*4]; host unshuffles.
"""

import sys

sys.path.insert(0, "/opt/trn_rl_repo")

from contextlib import ExitStack

import numpy as np

import concourse.bacc as bacc
import concourse.bass as bass
import concourse.tile as tile
from concourse import mybir
from concourse.bass import broadcast_tensor_aps
from concourse.bass_utils import run_bass_kernel_spmd
from concourse.masks import make_identity

F32 = mybir.dt.float32
F32R = mybir.dt.float32r
AF = mybir.ActivationFunctionType
ALU = mybir.AluOpType

B, N, D, K = 8, 65536, 256, 64
P = 128
BLK = 512                      # pixels per block
NIL = BLK // P                 # il count = 4

# config knobs
MM_F32R = True                 # f32r matmul (1cyc/row) vs fp32
X2_ON_ACT = 1                  # how many of the 4 x2 tiles go on Act
                               # (rest on DVE via scalar_tensor_tensor;
                               # tensor_tensor_reduce crashes TRN2 hw)


def build_nc(n: int = N, repeat: int = 1) -> bass.Bass:
    assert n % BLK == 0
    nblk = n // BLK
    nc = bacc.Bacc("TRN2", target_bir_lowering=False)
    # x natural bytes: [nblk, 4, 128, 256] = pixel (512b + 128il + r), d
    x = nc.declare_dram_parameter("x", [nblk, NIL, P, D], F32, isOutput=False)
    # g host-shuffled: [nblk, 128, 4, 64] = [b, r, il, k]
    g = nc.declare_dram_parameter("g", [nblk, P, NIL, K], F32, isOutput=False)
    c = nc.declare_dram_parameter("c", [K, D], F32, isOutput=False)
    iota = nc.declare_dram_parameter("iota", [P, K], F32, isOutput=False)
    # idx out: [r, b*4+il]
    out = nc.declare_dram_parameter("out", [P, nblk * NIL], F32, isOutput=True)

    with tile.TileContext(nc) as tc, ExitStack() as ctx:
        _body(ctx, tc, out, x, g, c, iota, nblk, repeat)
    nc.finalize()
    return nc


def _body(ctx, tc, out, x, g, c, iota, nblk, repeat=1):
    nc = tc.nc
    singles = ctx.enter_context(tc.tile_pool(name="singles", bufs=1))

    ident = singles.tile([P, P], F32)
    make_identity(nc, ident)

    # ---- preamble: ctn = -2*C^T (f32r), c2 column, iota ----
    c_sb = singles.tile([K, D], F32)
    nc.sync.dma_start(out=c_sb, in_=c[:, :])
    iota_sb = singles.tile([P, K], F32)
    nc.sync.dma_start(out=iota_sb, in_=iota[:, :])

    ctn = singles.tile([P, 2, K], F32R if MM_F32R else F32)
    c2_col = singles.tile([K, 1], F32)
    csq = singles.tile([K, D], F32)
    with tc.tile_pool(name="pre_ps", bufs=1, space="PSUM") as pre_ps:
        for j in range(2):
            ct_ps = pre_ps.tile([P, K], F32, tag=f"ct{j}")
            nc.tensor.transpose(ct_ps, c_sb[:, j * P:(j + 1) * P],
                                ident[0:K, 0:K])
            nc.scalar.mul(ctn[:, j, :], ct_ps, -2.0)
        nc.scalar.activation(csq, c_sb, AF.Square, accum_out=c2_col)

    idx_all = singles.tile([P, nblk * NIL], F32)

    # ---- pools ----
    xp = ctx.enter_context(tc.tile_pool(name="xp", bufs=3))
    gp = ctx.enter_context(tc.tile_pool(name="gp", bufs=3))
    xtp = ctx.enter_context(tc.tile_pool(name="xtp", bufs=2))
    tsbp = ctx.enter_context(tc.tile_pool(name="tsbp", bufs=2))
    scrp = ctx.enter_context(tc.tile_pool(name="scrp", bufs=2))
    ps_xt = ctx.enter_context(tc.tile_pool(name="ps_xt", bufs=2, space="PSUM"))
    ps_t = ctx.enter_context(tc.tile_pool(name="ps_t", bufs=2, space="PSUM"))
    ps_s = ctx.enter_context(tc.tile_pool(name="ps_s", bufs=2, space="PSUM"))

    rep_cm = tc.For_i(0, repeat) if repeat > 1 else None
    if rep_cm is not None:
        rep_cm.__enter__()
    for b in range(nblk):
        # ---- loads ----
        x_t = xp.tile([P, NIL, D], F32)
        nc.sync.dma_start(out=x_t, in_=x[b].rearrange("a b c -> b a c"))
        g_t = gp.tile([P, NIL, K], F32)
        nc.sync.dma_start(out=g_t, in_=g[b])

        # ---- x2 per tile: 1 on Act, 3 on DVE via stt (engine balance) ----
        x2_sb = scrp.tile([P, NIL, 1], F32, tag="x2")
        sq_scr = scrp.tile([P, NIL, D], F32, tag="sq")
        for il in range(NIL):
            if il < X2_ON_ACT:
                nc.scalar.activation(sq_scr[:, il, :], x_t[:, il, :],
                                     AF.Square, accum_out=x2_sb[:, il, :])
            else:
                nc.vector.scalar_tensor_tensor(
                    out=sq_scr[:, il, :], in0=x_t[:, il, :], scalar=0.0,
                    in1=x_t[:, il, :], op0=ALU.add, op1=ALU.mult,
                    accum_out=x2_sb[:, il, :])

        # ---- transposes in: x_t -> xt_ps [dmod, jc, il, r] ----
        xt_ps = ps_xt.tile([P, 2, NIL, P], F32)
        for jc in range(2):
            for il in range(NIL):
                nc.tensor.transpose(xt_ps[:, jc, il, :],
                                    x_t[:, il, jc * P:(jc + 1) * P], ident)
        xt_sb = xtp.tile([P, 2, NIL, P], F32R if MM_F32R else F32)
        nc.scalar.copy(xt_sb, xt_ps)

        # ---- matmul: t[k, (il,r)] = sum_jc ctn_jc^T @ xt_jc ----
        t_ps = ps_t.tile([K, NIL, P], F32)
        for jc in range(2):
            nc.tensor.matmul(t_ps, ctn[:, jc, :], xt_sb[:, jc, :, :],
                             start=(jc == 0), stop=(jc == 1))

        # ---- t_sb = t + c2 (psum -> sbuf, Act Identity with bias) ----
        t_sb = tsbp.tile([K, NIL, P], F32)
        nc.scalar.activation(t_sb, t_ps, AF.Identity, bias=c2_col, scale=1.0)

        # ---- transposes out: s_ps [r, il, k] ----
        s_ps = ps_s.tile([P, NIL, K], F32)
        for il in range(NIL):
            nc.tensor.transpose(s_ps[:, il, :], t_sb[:, il, :],
                                ident[0:K, 0:K])

        # ---- u = s_ps + x2 (broadcast over k), d = sqrt(u) ----
        u_sb = scrp.tile([P, NIL, K], F32, tag="u")
        in0_b, in1_b = broadcast_tensor_aps(s_ps[:, :, :], x2_sb[:, :, :])
        nc.vector.tensor_tensor(out=u_sb, in0=in0_b, in1=in1_b, op=ALU.add)
        d_sb = scrp.tile([P, NIL, K], F32, tag="d")
        nc.scalar.activation(d_sb, u_sb, AF.Sqrt)

        # ---- s = g - d, m = rowmax, idx = sum((s==m)*iota) ----
        # (tensor_tensor_reduce crashes TRN2 hw; use sub + reduce + stt)
        s_scr = scrp.tile([P, NIL, K], F32, tag="s")
        m_sb = scrp.tile([P, NIL], F32, tag="m")
        eq_scr = scrp.tile([P, NIL, K], F32, tag="eq")
        nc.vector.tensor_sub(s_scr, g_t, d_sb)
        nc.vector.tensor_reduce(out=m_sb, in_=s_scr,
                                axis=mybir.AxisListType.X, op=ALU.max)
        for il in range(NIL):
            nc.vector.scalar_tensor_tensor(
                out=eq_scr[:, il, :], in0=s_scr[:, il, :],
                scalar=m_sb[:, il:il + 1], in1=iota_sb,
                op0=ALU.is_equal, op1=ALU.mult,
                accum_out=idx_all[:, NIL * b + il:NIL * b + il + 1])

    if rep_cm is not None:
        rep_cm.__exit__(None, None, None)
    nc.sync.dma_start(out=out[:, :], in_=idx_all)


_NC_CACHE: dict[int, bass.Bass] = {}


def _get_nc(n: int) -> bass.Bass:
    if n not in _NC_CACHE:
        _NC_CACHE[n] = build_nc(n)
    return _NC_CACHE[n]


def _shuffle_g(gi: np.ndarray, n: int) -> np.ndarray:
    # [n, K] -> [nblk, P, NIL, K] with [b, r, il, k] = g[512b + 128il + r, k]
    nblk = n // BLK
    return np.ascontiguousarray(
        gi.reshape(nblk, NIL, P, K).transpose(0, 2, 1, 3))


def _unshuffle_idx(o: np.ndarray, n: int) -> np.ndarray:
    # [P, nblk*NIL] with [r, 4b+il] -> idx[512b + 128il + r]
    nblk = n // BLK
    return o.reshape(P, nblk, NIL).transpose(1, 2, 0).reshape(n)


def _execute(pixel_features, cluster_centers, gumbel_noise, **spmd_kwargs):
    b, n, d = pixel_features.shape
    k = cluster_centers.shape[1]
    assert (b, d, k) == (B, D, K)
    nblk = n // BLK
    nc = _get_nc(n)
    c0 = np.ascontiguousarray(cluster_centers[0], dtype=np.float32)
    iota = np.broadcast_to(np.arange(K, dtype=np.float32), (P, K)).copy()
    in_maps = []
    for i in range(b):
        xi = np.ascontiguousarray(pixel_features[i], dtype=np.float32)
        gi = np.ascontiguousarray(gumbel_noise[i], dtype=np.float32)
        in_maps.append({
            "x": xi.reshape(nblk, NIL, P, D),
            "g": _shuffle_g(gi, n),
            "c": c0,
            "iota": iota,
        })
    res = run_bass_kernel_spmd(nc, in_maps, list(range(b)), **spmd_kwargs)
    eye = np.eye(K, dtype=np.float32)
    outs = []
    for i in range(b):
        idx = _unshuffle_idx(np.asarray(res.results[i]["out"]), n)
        outs.append(eye[idx.astype(np.int64)])
    return np.stack(outs, axis=0), res


def kernel(pixel_features: np.ndarray, cluster_centers: np.ndarray,
           gumbel_noise: np.ndarray) -> np.ndarray:
    out, _ = _execute(pixel_features, cluster_centers, gumbel_noise)
    return out


if __name__ == "__main__":
    rng = np.random.default_rng(0)
    n = 1024
    xf = rng.standard_normal((B, n, D), dtype=np.float32)
    cc = rng.standard_normal((1, K, D), dtype=np.float32)
    gn = rng.standard_normal((B, n, K), dtype=np.float32)
    got = kernel(xf, cc, gn)
    d2 = (xf ** 2).sum(-1)[..., None] + (cc[0] ** 2).sum(-1)[None, None, :] \
        - 2.0 * np.einsum("bnd,kd->bnk", xf, cc[0])
    s = gn - np.sqrt(np.maximum(d2, 0.0))
    idx = s.argmax(-1)
    want = np.eye(K, dtype=np.float32)[idx]
    print("match:", (got.argmax(-1) == idx).mean(), "maxabs:",
          np.abs(got - want).max())


# revision 8
# speedup vs baseline: 1.1393x; 1.1393x over previous
"""Trainium2 Bass kernel v2 for DifferentiableHardKMeans (vq_codebook).

Math (per pixel p, cluster k):
    t[k,p]  = c2[k] - 2*c[k]@x[p]          (PE matmul, fp32r or fp32)
    u[p,k]  = t[p,k] + x2[p]
    s[p,k]  = g[p,k] - sqrt(u[p,k])
    idx[p]  = argmax_k s[p,k]
Output one-hot is expanded host-side from idx (exact 0.0/1.0 values,
matching the reference's y_hard - sg(y_soft) + y_soft identity).

Layout: per core (batch b -> core b), pixels processed in blocks of 512
(4 tiles of 128). Pixel i = 512*blk + 128*il + r lives on partition r,
free slot il. g is host-shuffled so each partition reads 1KB contiguous;
x is read with a strided AP (1KB runs). idx accumulates in SBUF and is
written once at the end as [128, nblk*4]; host unshuffles.
"""

import sys

sys.path.insert(0, "/opt/trn_rl_repo")

from contextlib import ExitStack

import numpy as np

import concourse.bacc as bacc
import concourse.bass as bass
import concourse.tile as tile
from concourse import mybir
from concourse.bass import broadcast_tensor_aps
from concourse.bass_utils import run_bass_kernel_spmd
from concourse.masks import make_identity

F32 = mybir.dt.float32
F32R = mybir.dt.float32r
AF = mybir.ActivationFunctionType
ALU = mybir.AluOpType

B, N, D, K = 8, 65536, 256, 64
P = 128
BLK = 512                      # pixels per block
NIL = BLK // P                 # il count = 4

# config knobs
MM_F32R = True                 # f32r matmul (1cyc/row) vs fp32
X2_ON_ACT = 1                  # how many of the 4 x2 tiles go on Act
                               # (rest on DVE via scalar_tensor_tensor;
                               # tensor_tensor_reduce crashes TRN2 hw)


def build_nc(n: int = N, repeat: int = 1) -> bass.Bass:
    assert n % BLK == 0
    nblk = n // BLK
    nc = bacc.Bacc("TRN2", target_bir_lowering=False)
    # x natural bytes: [nblk, 4, 128, 256] = pixel (512b + 128il + r), d
    x = nc.declare_dram_parameter("x", [nblk, NIL, P, D], F32, isOutput=False)
    # g host-shuffled: [nblk, 128, 4, 64] = [b, r, il, k]
    g = nc.declare_dram_parameter("g", [nblk, P, NIL, K], F32, isOutput=False)
    c = nc.declare_dram_parameter("c", [K, D], F32, isOutput=False)
    iota = nc.declare_dram_parameter("iota", [P, K], F32, isOutput=False)
    # idx out: [r, b*4+il]
    out = nc.declare_dram_parameter("out", [P, nblk * NIL], F32, isOutput=True)

    with tile.TileContext(nc) as tc, ExitStack() as ctx:
        _body(ctx, tc, out, x, g, c, iota, nblk, repeat)
    nc.finalize()
    return nc


def _body(ctx, tc, out, x, g, c, iota, nblk, repeat=1):
    nc = tc.nc
    singles = ctx.enter_context(tc.tile_pool(name="singles", bufs=1))

    ident = singles.tile([P, P], F32)
    make_identity(nc, ident)

    # ---- preamble: ctn = -2*C^T (f32r), c2 column, iota ----
    c_sb = singles.tile([K, D], F32)
    nc.sync.dma_start(out=c_sb, in_=c[:, :])
    iota_sb = singles.tile([P, K], F32)
    nc.sync.dma_start(out=iota_sb, in_=iota[:, :])

    ctn = singles.tile([P, 2, K], F32R if MM_F32R else F32)
    c2_col = singles.tile([K, 1], F32)
    csq = singles.tile([K, D], F32)
    with tc.tile_pool(name="pre_ps", bufs=1, space="PSUM") as pre_ps:
        for j in range(2):
            ct_ps = pre_ps.tile([P, K], F32, tag=f"ct{j}")
            nc.tensor.transpose(ct_ps, c_sb[:, j * P:(j + 1) * P],
                                ident[0:K, 0:K])
            nc.scalar.mul(ctn[:, j, :], ct_ps, -2.0)
        nc.scalar.activation(csq, c_sb, AF.Square, accum_out=c2_col)

    idx_all = singles.tile([P, nblk * NIL], F32)

    # ---- pools ----
    xp = ctx.enter_context(tc.tile_pool(name="xp", bufs=4))
    gp = ctx.enter_context(tc.tile_pool(name="gp", bufs=4))
    xtp = ctx.enter_context(tc.tile_pool(name="xtp", bufs=3))
    tsbp = ctx.enter_context(tc.tile_pool(name="tsbp", bufs=3))
    scrp = ctx.enter_context(tc.tile_pool(name="scrp", bufs=3))
    ps_xt = ctx.enter_context(tc.tile_pool(name="ps_xt", bufs=2, space="PSUM"))
    ps_t = ctx.enter_context(tc.tile_pool(name="ps_t", bufs=2, space="PSUM"))
    ps_s = ctx.enter_context(tc.tile_pool(name="ps_s", bufs=2, space="PSUM"))

    rep_cm = tc.For_i(0, repeat) if repeat > 1 else None
    if rep_cm is not None:
        rep_cm.__enter__()
    for b in range(nblk):
        # ---- loads ----
        x_t = xp.tile([P, NIL, D], F32)
        nc.sync.dma_start(out=x_t, in_=x[b].rearrange("a b c -> b a c"))
        g_t = gp.tile([P, NIL, K], F32)
        nc.scalar.dma_start(out=g_t, in_=g[b])

        # ---- x2 per tile: 1 on Act, 3 on DVE via stt (engine balance) ----
        x2_sb = scrp.tile([P, NIL, 1], F32, tag="x2")
        sq_scr = scrp.tile([P, NIL, D], F32, tag="sq")
        for il in range(NIL):
            if il < X2_ON_ACT:
                nc.scalar.activation(sq_scr[:, il, :], x_t[:, il, :],
                                     AF.Square, accum_out=x2_sb[:, il, :])
            else:
                nc.vector.scalar_tensor_tensor(
                    out=sq_scr[:, il, :], in0=x_t[:, il, :], scalar=0.0,
                    in1=x_t[:, il, :], op0=ALU.add, op1=ALU.mult,
                    accum_out=x2_sb[:, il, :])

        # ---- transposes in: x_t -> xt_ps [dmod, jc, il, r] ----
        xt_ps = ps_xt.tile([P, 2, NIL, P], F32)
        for jc in range(2):
            for il in range(NIL):
                nc.tensor.transpose(xt_ps[:, jc, il, :],
                                    x_t[:, il, jc * P:(jc + 1) * P], ident)
        xt_sb = xtp.tile([P, 2, NIL, P], F32R if MM_F32R else F32)
        nc.scalar.copy(xt_sb, xt_ps)

        # ---- matmul: t[k, (il,r)] = sum_jc ctn_jc^T @ xt_jc ----
        t_ps = ps_t.tile([K, NIL, P], F32)
        for jc in range(2):
            nc.tensor.matmul(t_ps, ctn[:, jc, :], xt_sb[:, jc, :, :],
                             start=(jc == 0), stop=(jc == 1))

        # ---- t_sb = t + c2 (psum -> sbuf, Act Identity with bias) ----
        t_sb = tsbp.tile([K, NIL, P], F32)
        nc.scalar.activation(t_sb, t_ps, AF.Identity, bias=c2_col, scale=1.0)

        # ---- transposes out: s_ps [r, il, k] ----
        s_ps = ps_s.tile([P, NIL, K], F32)
        for il in range(NIL):
            nc.tensor.transpose(s_ps[:, il, :], t_sb[:, il, :],
                                ident[0:K, 0:K])

        # ---- u = s_ps + x2 (broadcast over k), d = sqrt(u) ----
        u_sb = scrp.tile([P, NIL, K], F32, tag="u")
        in0_b, in1_b = broadcast_tensor_aps(s_ps[:, :, :], x2_sb[:, :, :])
        nc.vector.tensor_tensor(out=u_sb, in0=in0_b, in1=in1_b, op=ALU.add)
        d_sb = scrp.tile([P, NIL, K], F32, tag="d")
        nc.scalar.activation(d_sb, u_sb, AF.Sqrt)

        # ---- s = g - d, m = rowmax, idx = sum((s==m)*iota) ----
        # (tensor_tensor_reduce crashes TRN2 hw; use sub + reduce + stt)
        s_scr = scrp.tile([P, NIL, K], F32, tag="s")
        m_sb = scrp.tile([P, NIL], F32, tag="m")
        eq_scr = scrp.tile([P, NIL, K], F32, tag="eq")
        nc.vector.tensor_sub(s_scr, g_t, d_sb)
        nc.vector.tensor_reduce(out=m_sb, in_=s_scr,
                                axis=mybir.AxisListType.X, op=ALU.max)
        for il in range(NIL):
            nc.vector.scalar_tensor_tensor(
                out=eq_scr[:, il, :], in0=s_scr[:, il, :],
                scalar=m_sb[:, il:il + 1], in1=iota_sb,
                op0=ALU.is_equal, op1=ALU.mult,
                accum_out=idx_all[:, NIL * b + il:NIL * b + il + 1])

    if rep_cm is not None:
        rep_cm.__exit__(None, None, None)
    nc.sync.dma_start(out=out[:, :], in_=idx_all)


_NC_CACHE: dict[int, bass.Bass] = {}


def _get_nc(n: int) -> bass.Bass:
    if n not in _NC_CACHE:
        _NC_CACHE[n] = build_nc(n)
    return _NC_CACHE[n]


def _shuffle_g(gi: np.ndarray, n: int) -> np.ndarray:
    # [n, K] -> [nblk, P, NIL, K] with [b, r, il, k] = g[512b + 128il + r, k]
    nblk = n // BLK
    return np.ascontiguousarray(
        gi.reshape(nblk, NIL, P, K).transpose(0, 2, 1, 3))


def _unshuffle_idx(o: np.ndarray, n: int) -> np.ndarray:
    # [P, nblk*NIL] with [r, 4b+il] -> idx[512b + 128il + r]
    nblk = n // BLK
    return o.reshape(P, nblk, NIL).transpose(1, 2, 0).reshape(n)


def _execute(pixel_features, cluster_centers, gumbel_noise, **spmd_kwargs):
    b, n, d = pixel_features.shape
    k = cluster_centers.shape[1]
    assert (b, d, k) == (B, D, K)
    nblk = n // BLK
    nc = _get_nc(n)
    c0 = np.ascontiguousarray(cluster_centers[0], dtype=np.float32)
    iota = np.broadcast_to(np.arange(K, dtype=np.float32), (P, K)).copy()
    in_maps = []
    for i in range(b):
        xi = np.ascontiguousarray(pixel_features[i], dtype=np.float32)
        gi = np.ascontiguousarray(gumbel_noise[i], dtype=np.float32)
        in_maps.append({
            "x": xi.reshape(nblk, NIL, P, D),
            "g": _shuffle_g(gi, n),
            "c": c0,
            "iota": iota,
        })
    res = run_bass_kernel_spmd(nc, in_maps, list(range(b)), **spmd_kwargs)
    eye = np.eye(K, dtype=np.float32)
    outs = []
    for i in range(b):
        idx = _unshuffle_idx(np.asarray(res.results[i]["out"]), n)
        outs.append(eye[idx.astype(np.int64)])
    return np.stack(outs, axis=0), res


def kernel(pixel_features: np.ndarray, cluster_centers: np.ndarray,
           gumbel_noise: np.ndarray) -> np.ndarray:
    out, _ = _execute(pixel_features, cluster_centers, gumbel_noise)
    return out


if __name__ == "__main__":
    rng = np.random.default_rng(0)
    n = 1024
    xf = rng.standard_normal((B, n, D), dtype=np.float32)
    cc = rng.standard_normal((1, K, D), dtype=np.float32)
    gn = rng.standard_normal((B, n, K), dtype=np.float32)
    got = kernel(xf, cc, gn)
    d2 = (xf ** 2).sum(-1)[..., None] + (cc[0] ** 2).sum(-1)[None, None, :] \
        - 2.0 * np.einsum("bnd,kd->bnk", xf, cc[0])
    s = gn - np.sqrt(np.maximum(d2, 0.0))
    idx = s.argmax(-1)
    want = np.eye(K, dtype=np.float32)[idx]
    print("match:", (got.argmax(-1) == idx).mean(), "maxabs:",
          np.abs(got - want).max())
